# revision 44
# baseline (speedup 1.0000x reference)
"""DigitCaps u_hat kernel for Trainium2 (8 NeuronCores, SPMD).

Computes u_hat[b,r,c,o] = sum_i W[0,r,c,o,i] * x[b,r,i] + bias[o,0]
with B=512, R=1152, C=10, O=16, I=8 -> output [512, 1152, 10, 16, 1] f32.

Strategy
--------
Shard R across the 8 cores: 144 r-values per core; each core writes its
[512, 144, 160] f16 output slice (23.6 MB -- the kernel is output-DMA
bound at ~408 GB/s per core on the sync ring, so the whole game is
starting that stream early and never letting it stall).

Per group of G=3 r-values one matmul computes a [128 b, 480] tile:
  k = (r', i), i in [0,9)  (8 x-values + a constant-1 row for the bias)
  lhsT [27, 128] = x^T for a 128-wide b-block      (stationary)
  rhs  [27, 480] = block-diag W (3 x [9,160], bias row included)

PE row tiling: 4 groups ("a quad") sit at partition offsets
{0,32,64,96}; each matmul is K=27 in 32x128 tile mode, the four stream
concurrently through disjoint 32-row PE bands into 4 PSUM banks (quad
span ~590ns), and the pad rows 27..32 of each band are never read --
no zero-fill memsets at all (v1 padded K to 128 and spent ~20us of
memset+drain before the first output DMA).

PSUM: two 2-bank tiles per quad from DEDICATED tag rings (bufs=2 each)
so the tile-reuse chain spans two quads and neither the matmuls nor the
copies sit on the critical path (an untagged pool interleaves the rings
and serializes quad m's matmuls behind quad m-1's copies: 1.47us/quad
vs the 1.25us DMA floor).  Evacuation: DVE takes bands 0-1, ACT bands
2-3, in parallel ([128,2,480] f32->f16, 1.15/1.06us).

Queues: inputs (3.05 MB, DRAM-padded x+W chunks) on the gpsimd SWDGE
queue in FIFO-by-need order; outputs (24 x 0.98 MB) alone on the sync
HWDGE ring, which then runs gap-free at ~25.5 GB/s per SDMA engine
(~408 GB/s).  The dq-outer loop makes the first output DMAs depend
only on quads 0-1's inputs.  Things measured slower: outputs split
across two queues (ping-pong, 106us), small W-placement DMAs in the
sync ring ahead of outputs (receipt-serialized ~1.5us each -> 5us
mid-ramp stalls), compact W on side rings, consolidated single-sem
input DMAs; multi-level-partition-pattern DMA dests silently
mis-lower.  Best measured: 78.2us HW exec across 8 cores, rel err
3.3e-4 (baseline v1: 86.6us).
"""

import numpy as np

# Problem constants (hardcoded per harness contract).
B, R, C, O, I = 512, 1152, 10, 16, 8
CO = C * O                      # 160
NCORES = 8
RS = R // NCORES                # 144 r per core
G = 3                           # r-values per matmul (block-diag pack)
K = G * (I + 1)                 # 27 contraction rows (incl. bias row)
BANDS = 4                       # row-tiled matmuls per quad (PE 32x128 mode)
QUADS = RS // (G * BANDS)       # 12 quads per core
QPC = 2                         # quads per input chunk
CHUNKS = QUADS // QPC           # 6 input chunks (for early compute start)
XC = B                          # 512 x columns per quad slot
WC = G * CO                     # 480 W columns per quad slot
TC = XC + WC                    # 992 packed input columns
DMA_Q = 2                       # quads per output DMA (~1 MB transfers)
BBLK = B // 128                 # 4 b-blocks

OP_DT = "f16"                   # operand dtype (kept for test.py compat)
OUT_DT = "f16"                  # device output dtype

_prog_cache = {}


def _build_program(op_dt=OP_DT, out_dt=OUT_DT):
    import concourse.bacc as bacc
    import concourse.tile as tile
    from concourse import mybir

    key = (op_dt, out_dt)
    if key in _prog_cache:
        return _prog_cache[key]

    f32 = mybir.dt.float32
    f16 = mybir.dt.float16

    # Bacc (not raw Bass): its finalize() runs move_matmul_waits_to_ldweights
    # + generate_event_semaphores, required to satisfy the per-instruction
    # sync-wait limits at codegen.
    nc = bacc.Bacc("TRN2", target_bir_lowering=False, debug=False)

    in_d = nc.declare_dram_parameter(
        "inp", [CHUNKS, 128, QPC, TC], f16, isOutput=False
    )
    out_d = nc.declare_dram_parameter("out", [B, RS, CO], f16, isOutput=True)

    with tile.TileContext(nc) as tc:
        with (
            tc.tile_pool(name="const", bufs=1) as const,
            tc.tile_pool(name="psum", bufs=2, space="PSUM") as psum,
            tc.tile_pool(name="outp", bufs=6) as outp,
        ):
            # Input is 1.97 MB: x padded to the 32-row bands (1.57 MB,
            # chunked contiguous DMAs on the gpsimd SWDGE queue, quad 0
            # first -- the SDMA engines drain one queue's backlog before
            # another gets bandwidth, so the input's drain time gates the
            # output stream start), plus COMPACT W (0.40 MB): one dense
            # DVE memset zeroes wt (4x mode, done long before the first
            # CAST needs the engine), then 12 [9-partition] DMAs place
            # the non-zero blocks of each band's block-diagonal, split
            # across the sync+scalar rings which are idle until the
            # first output/copy.  (A multi-level partition-pattern DMA
            # would do this in 3 transfers but silently mis-lowers -
            # measured.)
            # All input loads (DRAM-padded x+W per chunk, zeros shipped
            # from HBM) on the gpsimd SWDGE queue: one FIFO queue drains
            # chunk 0 first at full rate, neither HWDGE sequencer is tied
            # up with input issues, and -- critically -- NOTHING sits in
            # the sync ring ahead of the output stream (small placement
            # DMAs there receipt-serialize ~1.5us each and stall the
            # outputs behind them: measured as 5us mid-ramp gaps).
            insb = []
            for ch in range(CHUNKS):
                t = const.tile([128, QPC, TC], f16, tag=f"in{ch}")
                if ch == 0:
                    for s in range(QPC):
                        nc.gpsimd.dma_start(out=t[:, s], in_=in_d[ch, :, s])
                else:
                    nc.gpsimd.dma_start(out=t[:], in_=in_d[ch])
                insb.append(t)

            # dq-outer, b-block-inner: the first 4 output DMAs need only
            # quads 0-1's inputs, so mid-stream input chunks are never on
            # the critical path.
            for dq in range(QUADS // DMA_Q):
                for j in range(BBLK):
                    ot = outp.tile([128, DMA_Q, BANDS, WC], f16)
                    for s2 in range(DMA_Q):
                        q = dq * DMA_Q + s2
                        ch, s = divmod(q, QPC)
                        # Two 2-bank psum tiles per quad with DEDICATED tag
                        # rings (bufs=2 each): tile reuse chains A(m)->A(m-2)
                        # give two quads of slack, so neither the matmuls nor
                        # the copies ever wait on the previous quad (the
                        # untagged pool interleaves A/B in one ring, which
                        # serializes quad m's matmuls behind quad m-1's
                        # copies -- measured 1.47us/quad vs the 1.25us DMA
                        # floor).
                        for h in range(2):
                            ps = psum.tile(
                                [128, 2, 512], f32, tag=f"ps{h}", bufs=2
                            )
                            for b2 in range(2):
                                band = 2 * h + b2
                                pb = 32 * band
                                lhsT = insb[ch][
                                    pb : pb + K, s, j * 128 : (j + 1) * 128
                                ]
                                rhs = insb[ch][pb : pb + K, s, XC : XC + WC]
                                # K=27 at partition offset pb: 32x128
                                # row-tile mode, 4 concurrent streams into 4
                                # banks.  Explicit tile_position: auto-derive
                                # rejects base partition 96.
                                nc.tensor.matmul(
                                    ps[:, b2, 0:WC], lhsT, rhs,
                                    start=True, stop=True,
                                    tile_position=(pb, 0),
                                )
                            if h == 0:
                                nc.vector.tensor_copy(
                                    ot[:, s2, 0:2, :], ps[:, :, 0:WC]
                                )
                            else:
                                nc.scalar.copy(
                                    ot[:, s2, 2:4, :], ps[:, :, 0:WC]
                                )
                    # All outputs on the sync ring (q1).  SDMA arbitration
                    # is strict-priority by queue index (q0>q1>q10,
                    # measured): outputs on q1 preempt the input tail on
                    # q10, while splitting outputs across q0+q1 ping-pongs
                    # (measured 106us).
                    nc.sync.dma_start(
                        out=out_d[
                            j * 128 : (j + 1) * 128,
                            dq * DMA_Q * G * BANDS : (dq + 1) * DMA_Q * G * BANDS,
                            :,
                        ],
                        in_=ot[:],
                    )

    nc.finalize()
    _prog_cache[key] = nc
    return nc


def _prep_inputs(x, W, bias, op_dt=OP_DT):
    """Build per-core packed input arrays in the device layout.

    Per chunk: [128, QPC, TC] f16 where partition p = 32*band + k,
    k = r'*9 + i (i=8 is the constant-1 bias row; rows 27..32 are pad),
    slot s picks the quad, cols [0:512] = x^T (b), cols [512:992] = the
    [27, 480] block-diag W for the band's group.
    """
    x = np.ascontiguousarray(x, dtype=np.float32)
    W = np.ascontiguousarray(W, dtype=np.float32)
    bias = np.ascontiguousarray(bias, dtype=np.float32)

    Wf = W[0].reshape(R, CO, I)                      # [R, CO, I]
    bias_co = np.tile(bias[:, 0], C)                 # [CO]
    NG = RS // G                                     # 48 groups per core

    in_maps = []
    for c in range(NCORES):
        r0 = c * RS

        xT = x[:, r0 : r0 + RS, :].transpose(1, 2, 0)    # [RS, I, B]
        seg9 = np.empty((RS, I + 1, B), dtype=np.float32)
        seg9[:, :I] = xT
        seg9[:, I] = 1.0
        g27 = seg9.reshape(NG, K, B)                     # rows k = r'*9+i
        arr = np.zeros((CHUNKS, BANDS, 32, QPC, TC), dtype=np.float16)
        # group g = (ch*QPC + s)*BANDS + band
        arr[:, :, :K, :, :XC] = (
            g27.reshape(CHUNKS, QPC, BANDS, K, B).transpose(0, 2, 3, 1, 4)
        )

        Wc = Wf[r0 : r0 + RS]                            # [RS, CO, I]
        W9 = np.empty((RS, I + 1, CO), dtype=np.float32)
        W9[:, :I] = Wc.transpose(0, 2, 1)
        W9[:, I] = bias_co
        blk = np.zeros((NG, G, I + 1, G, CO), dtype=np.float32)
        W9g = W9.reshape(NG, G, I + 1, CO)
        for rp in range(G):
            blk[:, rp, :, rp, :] = W9g[:, rp]
        blk27 = blk.reshape(NG, K, WC)
        arr[:, :, :K, :, XC:] = (
            blk27.reshape(CHUNKS, QPC, BANDS, K, WC).transpose(0, 2, 3, 1, 4)
        )

        in_maps.append({"inp": arr.reshape(CHUNKS, 128, QPC, TC)})
    return in_maps


def _run(inputs, trace=False, op_dt=OP_DT, out_dt=OUT_DT, **kw):
    from concourse.bass_utils import run_bass_kernel_spmd

    nc = _build_program(op_dt, out_dt)
    in_maps = _prep_inputs(inputs["x"], inputs["W"], inputs["bias"], op_dt)
    res = run_bass_kernel_spmd(
        nc, in_maps, list(range(NCORES)), trace=trace, **kw
    )
    outs = [np.asarray(res.results[c]["out"]) for c in range(NCORES)]
    full = np.concatenate(outs, axis=1)               # [B, R, CO]
    full = full.astype(np.float32, copy=False)
    return np.ascontiguousarray(full).reshape(B, R, C, O, 1), res


def kernel(x, W, bias):
    out, _ = _run({"x": x, "W": W, "bias": bias})
    return out
